# revision 13
# baseline (speedup 1.0000x reference)
"""MACE+Ewald forward on 8 Trainium2 NeuronCores.

Sharding: graph-per-core (8 graphs, 8 cores). Atoms balanced across 4 blocks
of <=128 slots (padded NL=512 per core); edges assigned to the core/block
owning their dst atom, packed into 128-edge tiles with per-block tile counts.

Key device-side structure per layer:
  1. hu = h @ Wup computed atom-major per block, AllGather (bf16, Shared out)
     kicked immediately so the Ewald block + radial-MLP prepass overlap it.
  2. Ewald: structure factors / he MLP, all bf16 matmuls.
  3. Radial MLP prepass for all edge tiles (gather-independent).
  4. Edge loop: batched indirect gather of hu rows per block; per tile the
     product-basis messages are scattered to dst atoms with matmuls whose
     moving operand is a host-precomputed segY matrix (the one-hot dst
     scatter matrix with the spherical harmonics Y and 1/avg_nei folded in),
     c-major output so no transposes are needed afterwards.
  5. Product basis (A^2 contractions) per block, h update, readout.

All heavy matmuls run with bf16 operands (4x PE throughput vs fp32; the
harness tolerance is 2e-2, measured error stays ~1e-3). segY ships as
fp8e4m3 to halve its DMA stream.
"""

import numpy as np
import ml_dtypes

C = 128
L = 2
NB = 8
NEL = 10
BG = 8
N_ATOMS = 3200
N_EDGES = 51200
R_MAX = 5.0
P_CUT = 5.0
AVG_NEI = 16.0
DELTA_K = 0.2
NKRBF = 128
DP = 8
SKIP = (2.0 + 1.0) ** -0.5
NL = 512            # padded atoms per core
NBLK = NL // 128    # atom blocks per core
KPAD = 128          # padded k-point count (real: 123)
LOFLM = np.repeat(np.arange(4), [1, 3, 5, 7])   # [16]
L_START = [0, 1, 4, 9]
L_WIDTH = [1, 3, 5, 7]
# scatter matmul chunks: (l, first lm, number of lm) with moving-free <= 512
CHUNKS = [(0, 0, 1), (1, 1, 3), (2, 4, 4), (2, 8, 1), (3, 9, 4), (3, 13, 3)]
SEGY_FP8 = True
HU_FP8 = False      # layer>=1 hu AllGather + gather in fp8e4m3

_CACHE = {}


# ---------------------------------------------------------------- host math
def _sph_np(u):
    x, y, z = u[:, 0], u[:, 1], u[:, 2]
    s3, s5, s15 = 3.0 ** 0.5, 5.0 ** 0.5, 15.0 ** 0.5
    c70, c105, c42, c7 = 70.0 ** 0.5 / 4.0, 105.0 ** 0.5, 42.0 ** 0.5 / 4.0, 7.0 ** 0.5 / 2.0
    comps = [np.ones_like(x),
             s3 * x, s3 * y, s3 * z,
             s15 * x * y, s15 * y * z, 0.5 * s5 * (3 * z * z - 1.0), s15 * x * z,
             0.5 * s15 * (x * x - y * y),
             c70 * y * (3 * x * x - y * y), c105 * x * y * z, c42 * y * (5 * z * z - 1.0),
             c7 * z * (5 * z * z - 3.0), c42 * x * (5 * z * z - 1.0),
             0.5 * c105 * z * (x * x - y * y), c70 * x * (x * x - 3 * y * y)]
    return np.stack(comps, axis=-1).astype(np.float32)


def _radial_np(r):
    n = np.arange(1, NB + 1, dtype=np.float32)
    rb = np.sqrt(2.0 / R_MAX) * np.sin(n * np.pi * r[:, None] / R_MAX) / np.maximum(r, 1e-9)[:, None]
    uu = np.clip(r / R_MAX, 0.0, 1.0)
    p = P_CUT
    env = 1.0 - (p + 1.0) * (p + 2.0) / 2.0 * uu ** 5 + p * (p + 2.0) * uu ** 6 - p * (p + 1.0) / 2.0 * uu ** 7
    env = env * (r < R_MAX)
    return (rb * env[:, None]).astype(np.float32)


def host_prep(inputs):
    """Build per-core padded arrays. Returns (in_maps, T_list, G4, e0)."""
    f32 = np.float32
    bf16 = ml_dtypes.bfloat16
    segy_np = ml_dtypes.float8_e4m3 if SEGY_FP8 else bf16
    pos = np.asarray(inputs['positions'], f32)
    attrs = np.asarray(inputs['node_attrs'], f32)
    shifts = np.asarray(inputs['shifts'], f32)
    eidx = np.asarray(inputs['edge_index']).astype(np.int64)
    batch = np.asarray(inputs['batch']).astype(np.int64)
    kgrid = np.asarray(inputs['kgrid'], f32)
    krbf = np.asarray(inputs['krbf'], f32)
    K = kgrid.shape[0]

    # per-graph contiguous atom ranges (batch is sorted)
    starts = np.searchsorted(batch, np.arange(BG))
    ends = np.searchsorted(batch, np.arange(BG), side='right')
    counts = ends - starts
    assert counts.max() <= NL, counts

    # balanced split of each graph's atoms into NBLK blocks of <=128 slots
    slot = np.zeros(N_ATOMS, np.int64)          # padded local slot per atom
    for b in range(BG):
        n = int(counts[b])
        base, rem = divmod(n, NBLK)
        sizes = [base + (k < rem) for k in range(NBLK)]
        assert max(sizes) <= 128
        cum = 0
        for k in range(NBLK):
            j = np.arange(cum, cum + sizes[k])
            slot[starts[b] + j] = k * 128 + (j - cum)
            cum += sizes[k]
    pid = (batch * NL + slot).astype(np.int32)  # padded global id [N]

    # ---- edge geometry (host) ----
    src, dst = eidx[0], eidx[1]
    vec = pos[dst] - pos[src] + shifts
    r = np.linalg.norm(vec.astype(np.float64), axis=1).astype(f32)
    uvec = vec / np.maximum(r, 1e-9)[:, None]
    Y = _sph_np(uvec)                           # [E,16]
    ef = _radial_np(r)                          # [E,8]

    # ---- Ewald geometry (host) ----
    dot = pos @ kgrid.T                         # [N,K]
    sd = np.prod(np.sinc(0.5 * DELTA_K * pos), axis=1).astype(f32)   # [N]
    cosd = (sd[:, None] * np.cos(dot)).astype(f32)
    sind = (sd[:, None] * np.sin(dot)).astype(f32)

    kdown = krbf @ np.asarray(inputs['Wdown'], f32)      # [K,DP]

    # ---- edge -> (core, block) assignment, per-block tile counts ----
    gdst = batch[dst]
    kblk = slot[dst] // 128
    ecount = np.zeros((BG, NBLK), np.int64)
    np.add.at(ecount, (gdst, kblk), 1)
    T_list = [max(1, int(np.ceil(ecount[:, k].max() / 128))) for k in range(NBLK)]
    O_list = np.concatenate([[0], np.cumsum(T_list)]).astype(int)
    NT = int(O_list[-1])
    G4 = ((NT + 3) // 4) * 4

    # ---- shared (replicated) weight arrays ----
    g = lambda k: np.asarray(inputs[k], f32)
    shared = {'Wembed': g('W_embed').astype(bf16),
              'ident': np.eye(128, dtype=f32).astype(bf16),
              'Wr0': g('Wr0').astype(bf16), 'Wr1a': g('Wr1a').astype(bf16),
              'Wr1b': g('Wr1b').astype(bf16)}
    # layer-0 hu is weight-only (h0 = attrs @ Wembed): precompute the full
    # gathered table on the host, killing the first AllGather.
    h0_full = attrs @ g('W_embed')                       # [N, C]
    huG0 = np.zeros((BG * NL, C), f32)
    huG0[pid] = h0_full @ g('Wup')[0]
    shared['huG0'] = huG0.astype(bf16)
    for i in range(L):
        for nm in ('Wpre1', 'Wpre2', 'Wm1', 'Wm2', 'Wup', 'Wmix'):
            shared[f'{nm}_{i}'] = g(nm)[i].astype(bf16)
        shared[f'rW1_{i}'] = g('rW1')[i].astype(bf16)
        shared[f'rW2_{i}'] = g('rW2')[i].astype(bf16)
        shared[f'rW3_{i}'] = g('rW3')[i].astype(bf16)
        # rW4 reshaped l-major: [64, l*128 + c]
        r4 = g('rW4')[i].reshape(64, C, 4).transpose(0, 2, 1).reshape(64, 4 * C)
        shared[f'rW4_{i}'] = r4.astype(bf16)
        for nm in ('bpre1', 'bpre2', 'bm1', 'bm2'):
            shared[f'{nm}_{i}'] = g(nm)[i].reshape(C, 1)
        for nm in ('rb1', 'rb2', 'rb3'):
            shared[f'{nm}_{i}'] = g(nm)[i].reshape(64, 1)
        kf = np.zeros((KPAD, C), f32)
        kf[:K] = 0.01 * (kdown @ g('WupE')[i])
        shared[f'kfilt_{i}'] = kf
        shared[f'w2T_{i}'] = g('w2')[i].T.copy()             # [C,4] f32
        shared[f'w3T_{i}'] = g('w3')[i].T.copy()

    # ---- per-core arrays ----
    in_maps = []
    for b in range(BG):
        nb = int(counts[b])
        sl = slice(starts[b], ends[b])
        m = dict(shared)
        slot_b = slot[sl]
        at = np.zeros((NEL, NL), f32)
        at[:, slot_b] = attrs[sl].T
        m['attrsT'] = at.astype(bf16)
        cam = np.zeros((128, NBLK * KPAD), f32)   # atom-major cosd, per block
        sam = np.zeros((128, NBLK * KPAD), f32)
        ckm = np.zeros((KPAD, NL), f32)           # k-major
        skm = np.zeros((KPAD, NL), f32)
        pr, bb = slot_b % 128, slot_b // 128
        cam.reshape(128, NBLK, KPAD)[pr, bb, :K] = cosd[sl]
        sam.reshape(128, NBLK, KPAD)[pr, bb, :K] = sind[sl]
        ckm[:K, slot_b] = cosd[sl].T
        skm[:K, slot_b] = sind[sl].T
        m['cosdam'], m['sindam'] = cam.astype(bf16), sam.astype(bf16)
        m['cosdkm'], m['sindkm'] = ckm.astype(bf16), skm.astype(bf16)

        efp = np.zeros((8, G4 * 128), f32)
        sip = np.zeros((128, NT), np.int32)
        segY = np.zeros((128, NT * 16 * 128), f32)
        emask = gdst == b
        for k in range(NBLK):
            es = np.nonzero(emask & (kblk == k))[0]
            es = es[np.argsort(slot[dst[es]], kind='stable')]
            s = np.arange(len(es))
            tt, p = s // 128, s % 128
            t = O_list[k] + tt
            efp[:, t * 128 + p] = ef[es].T
            sip[p, t] = pid[src[es]]
            a = slot[dst[es]] - k * 128
            base = t * 2048 + a
            for lm in range(16):
                segY[p, base + lm * 128] = Y[es, lm] / AVG_NEI
        m['efTpack'] = efp.astype(bf16)
        m['srcidx'] = sip
        m['segYpack'] = segY.astype(segy_np)
        in_maps.append(m)

    e0 = np.zeros(BG, f32)
    ae = attrs @ np.asarray(inputs['atomic_E'], f32)
    for b in range(BG):
        e0[b] = ae[starts[b]:ends[b]].sum()
    return in_maps, T_list, G4, e0


# ---------------------------------------------------------------- device
def build_kernel(T_list, G4):
    import concourse.bass as bass
    import concourse.bacc as bacc
    import concourse.mybir as mybir
    import concourse.tile as tile

    f32 = mybir.dt.float32
    bf16 = mybir.dt.bfloat16
    sdt = mybir.dt.float8e4 if SEGY_FP8 else bf16
    A = mybir.ActivationFunctionType
    NT = int(sum(T_list))
    Tmax = max(T_list)
    O_list = np.concatenate([[0], np.cumsum(T_list)]).astype(int)
    nc = bacc.Bacc("TRN2", target_bir_lowering=False, debug=False, num_devices=BG)

    dins = {}
    def din(name, shape, dt=f32):
        dins[name] = nc.dram_tensor(name, list(shape), dt, kind="ExternalInput").ap()
        return dins[name]

    # load order = SP queue order: the layer-0 critical path (srcidx for the
    # gathers, attrsT+Wembed for h, radial weights) goes first
    din('srcidx', (128, NT), mybir.dt.int32)
    din('attrsT', (NEL, NL), bf16)
    din('Wembed', (NEL, C), bf16)
    din('efTpack', (8, G4 * 128), bf16)
    segY_d = din('segYpack', (128, NT * 2048), sdt)
    huG0_d = din('huG0', (BG * NL, C), bf16)
    din('cosdam', (128, NBLK * KPAD), bf16); din('sindam', (128, NBLK * KPAD), bf16)
    din('cosdkm', (KPAD, NL), bf16); din('sindkm', (KPAD, NL), bf16)
    din('ident', (128, 128), bf16)
    din('Wr0', (C, 1), bf16); din('Wr1a', (C, 16), bf16); din('Wr1b', (16, 1), bf16)
    for i in range(L):
        for nm in ('Wpre1', 'Wpre2', 'Wm1', 'Wm2', 'Wup', 'Wmix'):
            din(f'{nm}_{i}', (C, C), bf16)
        din(f'rW1_{i}', (NB, 64), bf16); din(f'rW2_{i}', (64, 64), bf16)
        din(f'rW3_{i}', (64, 64), bf16); din(f'rW4_{i}', (64, 4 * C), bf16)
        for nm in ('bpre1', 'bpre2', 'bm1', 'bm2'):
            din(f'{nm}_{i}', (C, 1))
        for nm in ('rb1', 'rb2', 'rb3'):
            din(f'{nm}_{i}', (64, 1))
        din(f'kfilt_{i}', (KPAD, C))
        din(f'w2T_{i}', (C, 4)); din(f'w3T_{i}', (C, 4))
    en_out = nc.dram_tensor('en_out', [1, 1], f32, kind="ExternalOutput").ap()

    with tile.TileContext(nc) as tc:
        with (
            tc.tile_pool(name="const", bufs=1) as cp,
            tc.tile_pool(name="work", bufs=2) as wp,
            tc.tile_pool(name="segy", bufs=3) as sy,
            tc.tile_pool(name="big", bufs=1) as bp,
            tc.tile_pool(name="psA", bufs=1, space="PSUM") as psA,
            tc.tile_pool(name="psS", bufs=2, space="PSUM") as psS,
            tc.tile_pool(name="psW", bufs=2, space="PSUM") as psW,
            tc.tile_pool(name="dram", bufs=1, space="DRAM") as dp,
        ):
            sb = {}
            for name, ap in dins.items():
                if name in ('segYpack', 'huG0'):
                    continue            # DRAM-resident (streamed / gather source)
                t = cp.tile(list(ap.shape), ap.dtype, tag=name)
                nc.sync.dma_start(t[:], ap[:])
                sb[name] = t

            h = bp.tile([C, NL], bf16, tag="h")
            en = bp.tile([1, 1], f32, tag="en")
            feats_cm = bp.tile([C, NL], bf16, tag="feats_cm")
            s3sb = bp.tile([64, G4 * 128], bf16, tag="s3sb")
            hres_am = bp.tile([128, NBLK * 128], bf16, tag="hres_am")
            nc.vector.memset(en[:], 0.0)

            pe = psW.tile([C, NL], f32, tag="pb")
            nc.tensor.matmul(pe[:], sb['Wembed'][:], sb['attrsT'][:], start=True, stop=True)
            nc.scalar.activation(h[:], pe[:], A.Copy)

            hu_dt = mybir.dt.float8e4 if HU_FP8 else bf16
            coll = {}     # layer -> (huL, huG) for layers >= 1
            for i in range(L):
                # ---- gather source: host table (layer 0) or prior AllGather ----
                huG = huG0_d if i == 0 else coll[i][1][:]
                hug = wp.tile([128, NT * 128], bf16 if i == 0 else hu_dt,
                              tag=f"hug{i}", bufs=1)
                nc.gpsimd.indirect_dma_start(
                    out=hug[:], out_offset=None, in_=huG[:],
                    in_offset=bass.IndirectOffsetOnAxis(ap=sb['srcidx'][:], axis=0))
                if i + 1 < L:
                    huL_next = dp.tile([NL, C], hu_dt, tag=f"huL{i + 1}")
                    huG_next = dp.tile([BG * NL, C], hu_dt, tag=f"huG{i + 1}",
                                       addr_space="Shared")
                    hu_am = wp.tile([128, NL], hu_dt, tag="hu_am")
                    coll[i + 1] = (huL_next, huG_next)

                # ---- Ewald block (independent of the collective) ----
                p1 = psW.tile([C, NL], f32, tag="pb")
                nc.tensor.matmul(p1[:], sb[f'Wpre1_{i}'][:], h[:], start=True, stop=True)
                t1 = wp.tile([C, NL], bf16, tag="t1")
                nc.scalar.activation(t1[:], p1[:], A.Silu, bias=sb[f'bpre1_{i}'][:])
                p2 = psW.tile([C, NL], f32, tag="pb")
                nc.tensor.matmul(p2[:], sb[f'Wpre2_{i}'][:], t1[:], start=True, stop=True)
                hres = wp.tile([C, NL], bf16, tag="hres")
                nc.vector.tensor_scalar_add(hres[:], p2[:], sb[f'bpre2_{i}'][:])
                nc.vector.tensor_add(hres[:], hres[:], h[:])
                for k in range(NBLK):
                    pt = psS.tile([128, 512], f32, tag="ps")
                    ptb = pt[:].bitcast(bf16)[:, 0:128]
                    nc.tensor.transpose(ptb, hres[:, k * 128:(k + 1) * 128], sb['ident'][:])
                    nc.scalar.activation(hres_am[:, k * 128:(k + 1) * 128], ptb, A.Copy)
                sfk = {}
                for nm, am in (('r', 'cosdam'), ('i', 'sindam')):
                    psf = psS.tile([128, 512], f32, tag="ps")
                    for k in range(NBLK):
                        nc.tensor.matmul(psf[:, 0:128], sb[am][:, k * KPAD:(k + 1) * KPAD],
                                         hres_am[:, k * 128:(k + 1) * 128],
                                         start=(k == 0), stop=(k == NBLK - 1))
                    s = wp.tile([KPAD, C], bf16, tag=f"sfk{nm}")
                    nc.vector.tensor_tensor(s[:], psf[:, 0:128], sb[f'kfilt_{i}'][:],
                                            op=mybir.AluOpType.mult)
                    sfk[nm] = s
                phe = psW.tile([C, NL], f32, tag="pb")
                nc.tensor.matmul(phe[:], sfk['r'][:], sb['cosdkm'][:], start=True, stop=False)
                nc.tensor.matmul(phe[:], sfk['i'][:], sb['sindkm'][:], start=False, stop=True)
                he0 = wp.tile([C, NL], bf16, tag="he0")
                nc.scalar.activation(he0[:], phe[:], A.Copy)
                pm1 = psW.tile([C, NL], f32, tag="pb")
                nc.tensor.matmul(pm1[:], sb[f'Wm1_{i}'][:], he0[:], start=True, stop=True)
                tm = wp.tile([C, NL], bf16, tag="t1")
                nc.scalar.activation(tm[:], pm1[:], A.Silu, bias=sb[f'bm1_{i}'][:])
                pm2 = psW.tile([C, NL], f32, tag="pb")
                nc.tensor.matmul(pm2[:], sb[f'Wm2_{i}'][:], tm[:], start=True, stop=True)
                he2 = wp.tile([C, NL], bf16, tag="he2")
                nc.scalar.activation(he2[:], pm2[:], A.Silu, bias=sb[f'bm2_{i}'][:])

                # ---- radial MLP prepass (gather-independent) ----
                for gidx in range(G4 // 4):
                    gsl = slice(gidx * 512, (gidx + 1) * 512)
                    pr1 = psS.tile([128, 512], f32, tag="ps")
                    nc.tensor.matmul(pr1[0:64, :], sb[f'rW1_{i}'][:], sb['efTpack'][:, gsl],
                                     start=True, stop=True)
                    s1 = wp.tile([64, 512], bf16, tag="s1")
                    nc.scalar.activation(s1[:], pr1[0:64, :], A.Silu, bias=sb[f'rb1_{i}'][:])
                    pr2 = psS.tile([128, 512], f32, tag="ps")
                    nc.tensor.matmul(pr2[0:64, :], sb[f'rW2_{i}'][:], s1[:], start=True, stop=True)
                    s2 = wp.tile([64, 512], bf16, tag="s1")
                    nc.scalar.activation(s2[:], pr2[0:64, :], A.Silu, bias=sb[f'rb2_{i}'][:])
                    pr3 = psS.tile([128, 512], f32, tag="ps")
                    nc.tensor.matmul(pr3[0:64, :], sb[f'rW3_{i}'][:], s2[:], start=True, stop=True)
                    nc.scalar.activation(s3sb[:, gsl], pr3[0:64, :], A.Silu, bias=sb[f'rb3_{i}'][:])

                # ---- edge loop ----
                def issue_pw_mw(k, tt, i=i, hug=hug):
                    t = int(O_list[k]) + tt
                    sgt = sy.tile([128, 2048], sdt, tag="sg")
                    nc.sync.dma_start(sgt[:], segY_d[:, t * 2048:(t + 1) * 2048])
                    pw = psW.tile([C, NL], f32, tag="pb")
                    nc.tensor.matmul(pw[:], s3sb[:, t * 128:(t + 1) * 128],
                                     sb[f'rW4_{i}'][:], start=True, stop=True)
                    mw = wp.tile([128, 512], bf16, tag="mw")
                    nc.vector.tensor_tensor(
                        mw[:].rearrange("p (l c) -> p l c", l=4),
                        pw[:].rearrange("p (l c) -> p l c", l=4),
                        hug[:, t * 128:(t + 1) * 128].unsqueeze(1).broadcast_to([128, 4, 128]),
                        op=mybir.AluOpType.mult)
                    return mw, sgt

                def make_tail(k, i=i, he2=he2):
                    # layer tail for block k: h update, readout, next layer's hu.
                    # Deferred into the next block's tile stream so the product
                    # basis (DVE) overlaps the next block's scatters (PE).
                    def tail():
                        blk = slice(k * 128, (k + 1) * 128)
                        pmx = psW.tile([C, NL], f32, tag="pb")
                        nc.tensor.matmul(pmx[:, 0:128], sb[f'Wmix_{i}'][:],
                                         feats_cm[:, blk], start=True, stop=True)
                        hnk = wp.tile([C, 128], f32, tag="hn")
                        nc.vector.tensor_add(hnk[:], pmx[:, 0:128], he2[:, blk])
                        nc.vector.tensor_add(hnk[:], hnk[:], h[:, blk])
                        nc.vector.tensor_scalar_mul(h[:, blk], hnk[:], float(SKIP))
                        if i == 0:
                            prd = psS.tile([128, 512], f32, tag="ps")
                            nc.tensor.matmul(prd[0:1, 0:128], sb['Wr0'][:], h[:, blk],
                                             start=True, stop=True)
                            rs = wp.tile([1, 1], f32, tag="rs")
                            nc.vector.reduce_sum(rs[:], prd[0:1, 0:128],
                                                 axis=mybir.AxisListType.X)
                            nc.vector.tensor_add(en[:], en[:], rs[:])
                        else:
                            pra = psS.tile([128, 512], f32, tag="ps")
                            nc.tensor.matmul(pra[0:16, 0:128], sb['Wr1a'][:], h[:, blk],
                                             start=True, stop=True)
                            ta = wp.tile([16, 128], bf16, tag="ta")
                            nc.scalar.activation(ta[:], pra[0:16, 0:128], A.Silu)
                            prb = psS.tile([128, 512], f32, tag="ps")
                            nc.tensor.matmul(prb[0:1, 0:128], sb['Wr1b'][:], ta[:],
                                             start=True, stop=True)
                            rs = wp.tile([1, 1], f32, tag="rs")
                            nc.vector.reduce_sum(rs[:], prb[0:1, 0:128],
                                                 axis=mybir.AxisListType.X)
                            nc.vector.tensor_add(en[:], en[:], rs[:])
                        if i + 1 < L:
                            ph = psW.tile([C, NL], f32, tag="pb")
                            nc.tensor.matmul(ph[:, 0:128], h[:, blk],
                                             sb[f'Wup_{i + 1}'][:], start=True, stop=True)
                            nc.scalar.activation(hu_am[:, blk], ph[:, 0:128], A.Copy)
                            nc.sync.dma_start(huL_next[k * 128:(k + 1) * 128, :],
                                              hu_am[:, blk])
                    return tail

                look = None
                pending_tail = None
                for k in range(NBLK):
                    Tk = int(T_list[k])
                    pA = psA.tile([128, 2048], f32, tag="pA")
                    for tt in range(Tk):
                        if look is not None:
                            mw, sgt = look
                            look = None
                        else:
                            mw, sgt = issue_pw_mw(k, tt)
                        for (l, m0, w) in CHUNKS:
                            nc.tensor.matmul(pA[:, m0 * 128:(m0 + w) * 128],
                                             mw[:, l * 128:(l + 1) * 128],
                                             sgt[:, m0 * 128:(m0 + w) * 128],
                                             start=(tt == 0), stop=(tt == Tk - 1))
                        if pending_tail is not None and tt == min(3, Tk - 1):
                            pending_tail()
                            pending_tail = None
                    # free pA early: scal copy + A^2 on the scalar engine, then
                    # the DVE product basis reads SBUF only. The next block's
                    # first mw is issued ahead of it in the DVE queue.
                    scal = wp.tile([128, 128], bf16, tag="scal")
                    nc.scalar.activation(scal[:], pA[:, 0:128], A.Copy)
                    AA = wp.tile([128, 2048], bf16, tag="AA")
                    nc.scalar.activation(AA[:], pA[:], A.Square)
                    if k + 1 < NBLK:
                        look = issue_pw_mw(k + 1, 0)
                    # ---- product basis for this block (c-major throughout) ----
                    AA3 = AA[:].rearrange("c (m a) -> c a m", m=16)
                    inv = wp.tile([128, 512], f32, tag="inv")
                    for l in range(4):
                        nc.vector.reduce_sum(inv[:, l * 128:(l + 1) * 128].unsqueeze(2),
                                             AA3[:, :, L_START[l]:L_START[l] + L_WIDTH[l]],
                                             axis=mybir.AxisListType.X)
                    acc = {}
                    for wnm in ('w2T', 'w3T'):
                        t2 = wp.tile([128, 512], f32, tag="t2")
                        nc.vector.tensor_tensor(
                            t2[:].rearrange("c (l a) -> c l a", l=4),
                            inv[:].rearrange("c (l a) -> c l a", l=4),
                            sb[f'{wnm}_{i}'][:].unsqueeze(2).broadcast_to([128, 4, 128]),
                            op=mybir.AluOpType.mult)
                        ac = wp.tile([128, 128], f32, tag=f"ac{wnm}")
                        nc.vector.reduce_sum(ac[:].unsqueeze(2),
                                             t2[:].rearrange("c (l a) -> c a l", l=4),
                                             axis=mybir.AxisListType.X)
                        acc[wnm] = ac
                    fe = wp.tile([128, 128], f32, tag="fe")
                    nc.vector.tensor_tensor(fe[:], scal[:], acc['w3T'][:],
                                            op=mybir.AluOpType.mult)
                    nc.vector.tensor_add(fe[:], fe[:], acc['w2T'][:])
                    nc.vector.tensor_tensor(feats_cm[:, k * 128:(k + 1) * 128], fe[:],
                                            scal[:], op=mybir.AluOpType.add)
                    pending_tail = make_tail(k)
                pending_tail()
                if i + 1 < L:
                    nc.gpsimd.collective_compute(
                        "AllGather", mybir.AluOpType.bypass,
                        replica_groups=[list(range(BG))],
                        ins=[huL_next[:].opt()], outs=[huG_next[:].opt()])
            nc.sync.dma_start(en_out[:], en[:])
    nc.compile()
    return nc


def kernel(**inputs):
    from concourse import bass_utils
    in_maps, T_list, G4, e0 = host_prep(inputs)
    key = (tuple(T_list), G4)
    if key not in _CACHE:
        _CACHE[key] = build_kernel(T_list, G4)
    nc = _CACHE[key]
    res = bass_utils.run_bass_kernel_spmd(nc, in_maps, core_ids=list(range(BG)))
    energy = np.zeros(BG, np.float32)
    for b in range(BG):
        energy[b] = res.results[b]['en_out'].reshape(-1)[0] + e0[b]
    return energy


# revision 16
# speedup vs baseline: 1.0432x; 1.0432x over previous
"""MACE+Ewald forward on 8 Trainium2 NeuronCores.

Sharding: graph-per-core (8 graphs, 8 cores). Atoms balanced across 4 blocks
of <=128 slots (padded NL=512 per core); edges assigned to the core/block
owning their dst atom, packed into 128-edge tiles with per-block tile counts.

Key device-side structure per layer:
  1. hu = h @ Wup computed atom-major per block, AllGather (bf16, Shared out)
     kicked immediately so the Ewald block + radial-MLP prepass overlap it.
  2. Ewald: structure factors / he MLP, all bf16 matmuls.
  3. Radial MLP prepass for all edge tiles (gather-independent).
  4. Edge loop: batched indirect gather of hu rows per block; per tile the
     product-basis messages are scattered to dst atoms with matmuls whose
     moving operand is a host-precomputed segY matrix (the one-hot dst
     scatter matrix with the spherical harmonics Y and 1/avg_nei folded in),
     c-major output so no transposes are needed afterwards.
  5. Product basis (A^2 contractions) per block, h update, readout.

All heavy matmuls run with bf16 operands (4x PE throughput vs fp32; the
harness tolerance is 2e-2, measured error stays ~1e-3). segY ships as
fp8e4m3 to halve its DMA stream.
"""

import numpy as np
import ml_dtypes

C = 128
L = 2
NB = 8
NEL = 10
BG = 8
N_ATOMS = 3200
N_EDGES = 51200
R_MAX = 5.0
P_CUT = 5.0
AVG_NEI = 16.0
DELTA_K = 0.2
NKRBF = 128
DP = 8
SKIP = (2.0 + 1.0) ** -0.5
NL = 512            # padded atoms per core
NBLK = NL // 128    # atom blocks per core
KPAD = 128          # padded k-point count (real: 123)
LOFLM = np.repeat(np.arange(4), [1, 3, 5, 7])   # [16]
L_START = [0, 1, 4, 9]
L_WIDTH = [1, 3, 5, 7]
# scatter matmul chunks: (l, first lm, number of lm) with moving-free <= 512
CHUNKS = [(0, 0, 1), (1, 1, 3), (2, 4, 4), (2, 8, 1), (3, 9, 4), (3, 13, 3)]
SEGY_FP8 = True
HU_FP8 = False      # layer>=1 hu AllGather + gather in fp8e4m3

_CACHE = {}


# ---------------------------------------------------------------- host math
def _sph_np(u):
    x, y, z = u[:, 0], u[:, 1], u[:, 2]
    s3, s5, s15 = 3.0 ** 0.5, 5.0 ** 0.5, 15.0 ** 0.5
    c70, c105, c42, c7 = 70.0 ** 0.5 / 4.0, 105.0 ** 0.5, 42.0 ** 0.5 / 4.0, 7.0 ** 0.5 / 2.0
    comps = [np.ones_like(x),
             s3 * x, s3 * y, s3 * z,
             s15 * x * y, s15 * y * z, 0.5 * s5 * (3 * z * z - 1.0), s15 * x * z,
             0.5 * s15 * (x * x - y * y),
             c70 * y * (3 * x * x - y * y), c105 * x * y * z, c42 * y * (5 * z * z - 1.0),
             c7 * z * (5 * z * z - 3.0), c42 * x * (5 * z * z - 1.0),
             0.5 * c105 * z * (x * x - y * y), c70 * x * (x * x - 3 * y * y)]
    return np.stack(comps, axis=-1).astype(np.float32)


def _radial_np(r):
    n = np.arange(1, NB + 1, dtype=np.float32)
    rb = np.sqrt(2.0 / R_MAX) * np.sin(n * np.pi * r[:, None] / R_MAX) / np.maximum(r, 1e-9)[:, None]
    uu = np.clip(r / R_MAX, 0.0, 1.0)
    p = P_CUT
    env = 1.0 - (p + 1.0) * (p + 2.0) / 2.0 * uu ** 5 + p * (p + 2.0) * uu ** 6 - p * (p + 1.0) / 2.0 * uu ** 7
    env = env * (r < R_MAX)
    return (rb * env[:, None]).astype(np.float32)


def host_prep(inputs):
    """Build per-core padded arrays. Returns (in_maps, T_list, G4, e0)."""
    f32 = np.float32
    bf16 = ml_dtypes.bfloat16
    segy_np = ml_dtypes.float8_e4m3 if SEGY_FP8 else bf16
    pos = np.asarray(inputs['positions'], f32)
    attrs = np.asarray(inputs['node_attrs'], f32)
    shifts = np.asarray(inputs['shifts'], f32)
    eidx = np.asarray(inputs['edge_index']).astype(np.int64)
    batch = np.asarray(inputs['batch']).astype(np.int64)
    kgrid = np.asarray(inputs['kgrid'], f32)
    krbf = np.asarray(inputs['krbf'], f32)
    K = kgrid.shape[0]

    # per-graph contiguous atom ranges (batch is sorted)
    starts = np.searchsorted(batch, np.arange(BG))
    ends = np.searchsorted(batch, np.arange(BG), side='right')
    counts = ends - starts
    assert counts.max() <= NL, counts

    # balanced split of each graph's atoms into NBLK blocks of <=128 slots
    slot = np.zeros(N_ATOMS, np.int64)          # padded local slot per atom
    for b in range(BG):
        n = int(counts[b])
        base, rem = divmod(n, NBLK)
        sizes = [base + (k < rem) for k in range(NBLK)]
        assert max(sizes) <= 128
        cum = 0
        for k in range(NBLK):
            j = np.arange(cum, cum + sizes[k])
            slot[starts[b] + j] = k * 128 + (j - cum)
            cum += sizes[k]
    pid = (batch * NL + slot).astype(np.int32)  # padded global id [N]

    # ---- edge geometry (host) ----
    src, dst = eidx[0], eidx[1]
    vec = pos[dst] - pos[src] + shifts
    r = np.linalg.norm(vec.astype(np.float64), axis=1).astype(f32)
    uvec = vec / np.maximum(r, 1e-9)[:, None]
    Y = _sph_np(uvec)                           # [E,16]
    ef = _radial_np(r)                          # [E,8]

    # ---- Ewald geometry (host) ----
    dot = pos @ kgrid.T                         # [N,K]
    sd = np.prod(np.sinc(0.5 * DELTA_K * pos), axis=1).astype(f32)   # [N]
    cosd = (sd[:, None] * np.cos(dot)).astype(f32)
    sind = (sd[:, None] * np.sin(dot)).astype(f32)

    kdown = krbf @ np.asarray(inputs['Wdown'], f32)      # [K,DP]

    # ---- edge -> (core, block) assignment, per-block tile counts ----
    gdst = batch[dst]
    kblk = slot[dst] // 128
    ecount = np.zeros((BG, NBLK), np.int64)
    np.add.at(ecount, (gdst, kblk), 1)
    T_list = [max(1, int(np.ceil(ecount[:, k].max() / 128))) for k in range(NBLK)]
    O_list = np.concatenate([[0], np.cumsum(T_list)]).astype(int)
    NT = int(O_list[-1])
    G4 = ((NT + 3) // 4) * 4

    # ---- shared (replicated) weight arrays ----
    g = lambda k: np.asarray(inputs[k], f32)
    shared = {'Wembed': g('W_embed').astype(bf16),
              'ident': np.eye(128, dtype=f32).astype(bf16),
              'Wr0': g('Wr0').astype(bf16), 'Wr1a': g('Wr1a').astype(bf16),
              'Wr1b': g('Wr1b').astype(bf16)}
    # layer-0 hu is weight-only (h0 = attrs @ Wembed): precompute the full
    # gathered table on the host, killing the first AllGather.
    h0_full = attrs @ g('W_embed')                       # [N, C]
    huG0 = np.zeros((BG * NL, C), f32)
    huG0[pid] = h0_full @ g('Wup')[0]
    shared['huG0'] = huG0.astype(bf16)
    for i in range(L):
        for nm in ('Wpre1', 'Wpre2', 'Wm1', 'Wm2', 'Wup', 'Wmix'):
            shared[f'{nm}_{i}'] = g(nm)[i].astype(bf16)
        shared[f'rW1_{i}'] = g('rW1')[i].astype(bf16)
        shared[f'rW2_{i}'] = g('rW2')[i].astype(bf16)
        shared[f'rW3_{i}'] = g('rW3')[i].astype(bf16)
        # rW4 reshaped l-major: [64, l*128 + c]
        r4 = g('rW4')[i].reshape(64, C, 4).transpose(0, 2, 1).reshape(64, 4 * C)
        shared[f'rW4_{i}'] = r4.astype(bf16)
        for nm in ('bpre1', 'bpre2', 'bm1', 'bm2'):
            shared[f'{nm}_{i}'] = g(nm)[i].reshape(C, 1)
        for nm in ('rb1', 'rb2', 'rb3'):
            shared[f'{nm}_{i}'] = g(nm)[i].reshape(64, 1)
        kf = np.zeros((KPAD, C), f32)
        kf[:K] = 0.01 * (kdown @ g('WupE')[i])
        shared[f'kfilt_{i}'] = kf
        shared[f'w2T_{i}'] = g('w2')[i].T.copy()             # [C,4] f32
        shared[f'w3T_{i}'] = g('w3')[i].T.copy()

    # ---- per-core arrays ----
    in_maps = []
    for b in range(BG):
        nb = int(counts[b])
        sl = slice(starts[b], ends[b])
        m = dict(shared)
        slot_b = slot[sl]
        at = np.zeros((NEL, NL), f32)
        at[:, slot_b] = attrs[sl].T
        m['attrsT'] = at.astype(bf16)
        cam = np.zeros((128, NBLK * KPAD), f32)   # atom-major cosd, per block
        sam = np.zeros((128, NBLK * KPAD), f32)
        ckm = np.zeros((KPAD, NL), f32)           # k-major
        skm = np.zeros((KPAD, NL), f32)
        pr, bb = slot_b % 128, slot_b // 128
        cam.reshape(128, NBLK, KPAD)[pr, bb, :K] = cosd[sl]
        sam.reshape(128, NBLK, KPAD)[pr, bb, :K] = sind[sl]
        ckm[:K, slot_b] = cosd[sl].T
        skm[:K, slot_b] = sind[sl].T
        m['cosdam'], m['sindam'] = cam.astype(bf16), sam.astype(bf16)
        m['cosdkm'], m['sindkm'] = ckm.astype(bf16), skm.astype(bf16)

        efp = np.zeros((8, G4 * 128), f32)
        sip = np.zeros((128, NT), np.int32)
        segY = np.zeros((128, NT * 16 * 128), f32)
        emask = gdst == b
        for k in range(NBLK):
            es = np.nonzero(emask & (kblk == k))[0]
            es = es[np.argsort(slot[dst[es]], kind='stable')]
            s = np.arange(len(es))
            tt, p = s // 128, s % 128
            t = O_list[k] + tt
            efp[:, t * 128 + p] = ef[es].T
            sip[p, t] = pid[src[es]]
            a = slot[dst[es]] - k * 128
            base = t * 2048 + a
            for lm in range(16):
                segY[p, base + lm * 128] = Y[es, lm] / AVG_NEI
        m['efTpack'] = efp.astype(bf16)
        m['srcidx'] = sip
        m['segYpack'] = segY.astype(segy_np)
        in_maps.append(m)

    e0 = np.zeros(BG, f32)
    ae = attrs @ np.asarray(inputs['atomic_E'], f32)
    for b in range(BG):
        e0[b] = ae[starts[b]:ends[b]].sum()
    return in_maps, T_list, G4, e0


# ---------------------------------------------------------------- device
def build_kernel(T_list, G4):
    import concourse.bass as bass
    import concourse.bacc as bacc
    import concourse.mybir as mybir
    import concourse.tile as tile

    f32 = mybir.dt.float32
    bf16 = mybir.dt.bfloat16
    sdt = mybir.dt.float8e4 if SEGY_FP8 else bf16
    A = mybir.ActivationFunctionType
    NT = int(sum(T_list))
    Tmax = max(T_list)
    O_list = np.concatenate([[0], np.cumsum(T_list)]).astype(int)
    nc = bacc.Bacc("TRN2", target_bir_lowering=False, debug=False, num_devices=BG)

    dins = {}
    def din(name, shape, dt=f32):
        dins[name] = nc.dram_tensor(name, list(shape), dt, kind="ExternalInput").ap()
        return dins[name]

    # load order = SP queue order: the layer-0 critical path (srcidx for the
    # gathers, attrsT+Wembed for h, radial weights) goes first
    din('srcidx', (128, NT), mybir.dt.int32)
    din('attrsT', (NEL, NL), bf16)
    din('Wembed', (NEL, C), bf16)
    din('efTpack', (8, G4 * 128), bf16)
    segY_d = din('segYpack', (128, NT * 2048), sdt)
    huG0_d = din('huG0', (BG * NL, C), bf16)
    din('cosdam', (128, NBLK * KPAD), bf16); din('sindam', (128, NBLK * KPAD), bf16)
    din('cosdkm', (KPAD, NL), bf16); din('sindkm', (KPAD, NL), bf16)
    din('ident', (128, 128), bf16)
    din('Wr0', (C, 1), bf16); din('Wr1a', (C, 16), bf16); din('Wr1b', (16, 1), bf16)
    for i in range(L):
        for nm in ('Wpre1', 'Wpre2', 'Wm1', 'Wm2', 'Wup', 'Wmix'):
            din(f'{nm}_{i}', (C, C), bf16)
        din(f'rW1_{i}', (NB, 64), bf16); din(f'rW2_{i}', (64, 64), bf16)
        din(f'rW3_{i}', (64, 64), bf16); din(f'rW4_{i}', (64, 4 * C), bf16)
        for nm in ('bpre1', 'bpre2', 'bm1', 'bm2'):
            din(f'{nm}_{i}', (C, 1))
        for nm in ('rb1', 'rb2', 'rb3'):
            din(f'{nm}_{i}', (64, 1))
        din(f'kfilt_{i}', (KPAD, C))
        din(f'w2T_{i}', (C, 4)); din(f'w3T_{i}', (C, 4))
    en_out = nc.dram_tensor('en_out', [1, 1], f32, kind="ExternalOutput").ap()

    with tile.TileContext(nc) as tc:
        with (
            tc.tile_pool(name="const", bufs=1) as cp,
            tc.tile_pool(name="work", bufs=2) as wp,
            tc.tile_pool(name="segy", bufs=3) as sy,
            tc.tile_pool(name="big", bufs=1) as bp,
            tc.tile_pool(name="psA", bufs=1, space="PSUM") as psA,
            tc.tile_pool(name="psS", bufs=2, space="PSUM") as psS,
            tc.tile_pool(name="psW", bufs=2, space="PSUM") as psW,
            tc.tile_pool(name="dram", bufs=1, space="DRAM") as dp,
        ):
            sb = {}
            for name, ap in dins.items():
                if name in ('segYpack', 'huG0'):
                    continue            # DRAM-resident (streamed / gather source)
                t = cp.tile(list(ap.shape), ap.dtype, tag=name)
                nc.sync.dma_start(t[:], ap[:])
                sb[name] = t

            h = bp.tile([C, NL], bf16, tag="h")
            en = bp.tile([1, 1], f32, tag="en")
            feats_cm = bp.tile([C, NL], bf16, tag="feats_cm")
            s3sb = bp.tile([64, G4 * 128], bf16, tag="s3sb")
            hres_am = bp.tile([128, NBLK * 128], bf16, tag="hres_am")
            nc.vector.memset(en[:], 0.0)

            pe = psW.tile([C, NL], f32, tag="pb")
            nc.tensor.matmul(pe[:], sb['Wembed'][:], sb['attrsT'][:], start=True, stop=True)
            nc.scalar.activation(h[:], pe[:], A.Copy)

            hu_dt = mybir.dt.float8e4 if HU_FP8 else bf16
            coll = {}     # layer -> (huL, huG) for layers >= 1
            for i in range(L):
                # ---- gather source: host table (layer 0) or prior AllGather ----
                huG = huG0_d if i == 0 else coll[i][1][:]
                hugs = []
                for k in range(NBLK):
                    lo, hi = int(O_list[k]), int(O_list[k + 1])
                    hg = wp.tile([128, Tmax * 128], bf16 if i == 0 else hu_dt,
                                 tag=f"hug{k % 2}{i}", bufs=1)
                    nc.gpsimd.indirect_dma_start(
                        out=hg[:, 0:(hi - lo) * 128], out_offset=None, in_=huG[:],
                        in_offset=bass.IndirectOffsetOnAxis(
                            ap=sb['srcidx'][:, lo:hi], axis=0))
                    hugs.append(hg)
                if i + 1 < L:
                    huL_next = dp.tile([NL, C], hu_dt, tag=f"huL{i + 1}")
                    huG_next = dp.tile([BG * NL, C], hu_dt, tag=f"huG{i + 1}",
                                       addr_space="Shared")
                    hu_am = wp.tile([128, NL], hu_dt, tag="hu_am")
                    coll[i + 1] = (huL_next, huG_next)

                # ---- Ewald block (independent of the collective) ----
                p1 = psW.tile([C, NL], f32, tag="pb")
                nc.tensor.matmul(p1[:], sb[f'Wpre1_{i}'][:], h[:], start=True, stop=True)
                t1 = wp.tile([C, NL], bf16, tag="t1")
                nc.scalar.activation(t1[:], p1[:], A.Silu, bias=sb[f'bpre1_{i}'][:])
                p2 = psW.tile([C, NL], f32, tag="pb")
                nc.tensor.matmul(p2[:], sb[f'Wpre2_{i}'][:], t1[:], start=True, stop=True)
                hres = wp.tile([C, NL], bf16, tag="hres")
                nc.vector.tensor_scalar_add(hres[:], p2[:], sb[f'bpre2_{i}'][:])
                nc.vector.tensor_add(hres[:], hres[:], h[:])
                for k in range(NBLK):
                    pt = psS.tile([128, 512], f32, tag="ps")
                    ptb = pt[:].bitcast(bf16)[:, 0:128]
                    nc.tensor.transpose(ptb, hres[:, k * 128:(k + 1) * 128], sb['ident'][:])
                    nc.scalar.activation(hres_am[:, k * 128:(k + 1) * 128], ptb, A.Copy)
                sfk = {}
                for nm, am in (('r', 'cosdam'), ('i', 'sindam')):
                    psf = psS.tile([128, 512], f32, tag="ps")
                    for k in range(NBLK):
                        nc.tensor.matmul(psf[:, 0:128], sb[am][:, k * KPAD:(k + 1) * KPAD],
                                         hres_am[:, k * 128:(k + 1) * 128],
                                         start=(k == 0), stop=(k == NBLK - 1))
                    s = wp.tile([KPAD, C], bf16, tag=f"sfk{nm}")
                    nc.vector.tensor_tensor(s[:], psf[:, 0:128], sb[f'kfilt_{i}'][:],
                                            op=mybir.AluOpType.mult)
                    sfk[nm] = s
                phe = psW.tile([C, NL], f32, tag="pb")
                nc.tensor.matmul(phe[:], sfk['r'][:], sb['cosdkm'][:], start=True, stop=False)
                nc.tensor.matmul(phe[:], sfk['i'][:], sb['sindkm'][:], start=False, stop=True)
                he0 = wp.tile([C, NL], bf16, tag="he0")
                nc.scalar.activation(he0[:], phe[:], A.Copy)
                pm1 = psW.tile([C, NL], f32, tag="pb")
                nc.tensor.matmul(pm1[:], sb[f'Wm1_{i}'][:], he0[:], start=True, stop=True)
                tm = wp.tile([C, NL], bf16, tag="t1")
                nc.scalar.activation(tm[:], pm1[:], A.Silu, bias=sb[f'bm1_{i}'][:])
                pm2 = psW.tile([C, NL], f32, tag="pb")
                nc.tensor.matmul(pm2[:], sb[f'Wm2_{i}'][:], tm[:], start=True, stop=True)
                he2 = wp.tile([C, NL], bf16, tag="he2")
                nc.scalar.activation(he2[:], pm2[:], A.Silu, bias=sb[f'bm2_{i}'][:])

                # ---- radial MLP prepass (gather-independent) ----
                for gidx in range(G4 // 4):
                    gsl = slice(gidx * 512, (gidx + 1) * 512)
                    pr1 = psS.tile([128, 512], f32, tag="ps")
                    nc.tensor.matmul(pr1[0:64, :], sb[f'rW1_{i}'][:], sb['efTpack'][:, gsl],
                                     start=True, stop=True)
                    s1 = wp.tile([64, 512], bf16, tag="s1")
                    nc.scalar.activation(s1[:], pr1[0:64, :], A.Silu, bias=sb[f'rb1_{i}'][:])
                    pr2 = psS.tile([128, 512], f32, tag="ps")
                    nc.tensor.matmul(pr2[0:64, :], sb[f'rW2_{i}'][:], s1[:], start=True, stop=True)
                    s2 = wp.tile([64, 512], bf16, tag="s1")
                    nc.scalar.activation(s2[:], pr2[0:64, :], A.Silu, bias=sb[f'rb2_{i}'][:])
                    pr3 = psS.tile([128, 512], f32, tag="ps")
                    nc.tensor.matmul(pr3[0:64, :], sb[f'rW3_{i}'][:], s2[:], start=True, stop=True)
                    nc.scalar.activation(s3sb[:, gsl], pr3[0:64, :], A.Silu, bias=sb[f'rb3_{i}'][:])

                # ---- edge loop ----
                def issue_pw_mw(k, tt, i=i, hugs=hugs):
                    t = int(O_list[k]) + tt
                    sgt = sy.tile([128, 2048], sdt, tag="sg")
                    nc.sync.dma_start(sgt[:], segY_d[:, t * 2048:(t + 1) * 2048])
                    pw = psW.tile([C, NL], f32, tag="pb")
                    nc.tensor.matmul(pw[:], s3sb[:, t * 128:(t + 1) * 128],
                                     sb[f'rW4_{i}'][:], start=True, stop=True)
                    mw = wp.tile([128, 512], bf16, tag="mw")
                    nc.vector.tensor_tensor(
                        mw[:].rearrange("p (l c) -> p l c", l=4),
                        pw[:].rearrange("p (l c) -> p l c", l=4),
                        hugs[k][:, tt * 128:(tt + 1) * 128].unsqueeze(1).broadcast_to([128, 4, 128]),
                        op=mybir.AluOpType.mult)
                    return mw, sgt

                def make_tail(k, i=i, he2=he2):
                    # layer tail for block k: h update, readout, next layer's hu.
                    # Deferred into the next block's tile stream so the product
                    # basis (DVE) overlaps the next block's scatters (PE).
                    def tail():
                        blk = slice(k * 128, (k + 1) * 128)
                        pmx = psW.tile([C, NL], f32, tag="pb")
                        nc.tensor.matmul(pmx[:, 0:128], sb[f'Wmix_{i}'][:],
                                         feats_cm[:, blk], start=True, stop=True)
                        hnk = wp.tile([C, 128], f32, tag="hn")
                        nc.vector.tensor_add(hnk[:], pmx[:, 0:128], he2[:, blk])
                        nc.vector.tensor_add(hnk[:], hnk[:], h[:, blk])
                        nc.vector.tensor_scalar_mul(h[:, blk], hnk[:], float(SKIP))
                        if i == 0:
                            prd = psS.tile([128, 512], f32, tag="ps")
                            nc.tensor.matmul(prd[0:1, 0:128], sb['Wr0'][:], h[:, blk],
                                             start=True, stop=True)
                            rs = wp.tile([1, 1], f32, tag="rs")
                            nc.vector.reduce_sum(rs[:], prd[0:1, 0:128],
                                                 axis=mybir.AxisListType.X)
                            nc.vector.tensor_add(en[:], en[:], rs[:])
                        else:
                            pra = psS.tile([128, 512], f32, tag="ps")
                            nc.tensor.matmul(pra[0:16, 0:128], sb['Wr1a'][:], h[:, blk],
                                             start=True, stop=True)
                            ta = wp.tile([16, 128], bf16, tag="ta")
                            nc.scalar.activation(ta[:], pra[0:16, 0:128], A.Silu)
                            prb = psS.tile([128, 512], f32, tag="ps")
                            nc.tensor.matmul(prb[0:1, 0:128], sb['Wr1b'][:], ta[:],
                                             start=True, stop=True)
                            rs = wp.tile([1, 1], f32, tag="rs")
                            nc.vector.reduce_sum(rs[:], prb[0:1, 0:128],
                                                 axis=mybir.AxisListType.X)
                            nc.vector.tensor_add(en[:], en[:], rs[:])
                        if i + 1 < L:
                            ph = psW.tile([C, NL], f32, tag="pb")
                            nc.tensor.matmul(ph[:, 0:128], h[:, blk],
                                             sb[f'Wup_{i + 1}'][:], start=True, stop=True)
                            nc.scalar.activation(hu_am[:, blk], ph[:, 0:128], A.Copy)
                            nc.sync.dma_start(huL_next[k * 128:(k + 1) * 128, :],
                                              hu_am[:, blk])
                    return tail

                look = None
                pending_tail = None
                for k in range(NBLK):
                    Tk = int(T_list[k])
                    pA = psA.tile([128, 2048], f32, tag="pA")
                    for tt in range(Tk):
                        if look is not None:
                            mw, sgt = look
                            look = None
                        else:
                            mw, sgt = issue_pw_mw(k, tt)
                        for (l, m0, w) in CHUNKS:
                            nc.tensor.matmul(pA[:, m0 * 128:(m0 + w) * 128],
                                             mw[:, l * 128:(l + 1) * 128],
                                             sgt[:, m0 * 128:(m0 + w) * 128],
                                             start=(tt == 0), stop=(tt == Tk - 1))
                        if pending_tail is not None and tt == min(3, Tk - 1):
                            pending_tail()
                            pending_tail = None
                    # free pA early: scal copy + A^2 on the scalar engine, then
                    # the DVE product basis reads SBUF only. The next block's
                    # first mw is issued ahead of it in the DVE queue.
                    scal = wp.tile([128, 128], bf16, tag="scal")
                    nc.scalar.activation(scal[:], pA[:, 0:128], A.Copy)
                    AA = wp.tile([128, 2048], bf16, tag="AA")
                    nc.scalar.activation(AA[:], pA[:], A.Square)
                    if k + 1 < NBLK:
                        look = issue_pw_mw(k + 1, 0)
                    # ---- product basis for this block (c-major throughout) ----
                    AA3 = AA[:].rearrange("c (m a) -> c a m", m=16)
                    inv = wp.tile([128, 512], f32, tag="inv")
                    for l in range(4):
                        nc.vector.reduce_sum(inv[:, l * 128:(l + 1) * 128].unsqueeze(2),
                                             AA3[:, :, L_START[l]:L_START[l] + L_WIDTH[l]],
                                             axis=mybir.AxisListType.X)
                    acc = {}
                    for wnm in ('w2T', 'w3T'):
                        t2 = wp.tile([128, 512], f32, tag="t2")
                        nc.vector.tensor_tensor(
                            t2[:].rearrange("c (l a) -> c l a", l=4),
                            inv[:].rearrange("c (l a) -> c l a", l=4),
                            sb[f'{wnm}_{i}'][:].unsqueeze(2).broadcast_to([128, 4, 128]),
                            op=mybir.AluOpType.mult)
                        ac = wp.tile([128, 128], f32, tag=f"ac{wnm}")
                        nc.vector.reduce_sum(ac[:].unsqueeze(2),
                                             t2[:].rearrange("c (l a) -> c a l", l=4),
                                             axis=mybir.AxisListType.X)
                        acc[wnm] = ac
                    fe = wp.tile([128, 128], f32, tag="fe")
                    nc.vector.tensor_tensor(fe[:], scal[:], acc['w3T'][:],
                                            op=mybir.AluOpType.mult)
                    nc.vector.tensor_add(fe[:], fe[:], acc['w2T'][:])
                    nc.vector.tensor_tensor(feats_cm[:, k * 128:(k + 1) * 128], fe[:],
                                            scal[:], op=mybir.AluOpType.add)
                    pending_tail = make_tail(k)
                pending_tail()
                if i + 1 < L:
                    nc.gpsimd.collective_compute(
                        "AllGather", mybir.AluOpType.bypass,
                        replica_groups=[list(range(BG))],
                        ins=[huL_next[:].opt()], outs=[huG_next[:].opt()])
            nc.sync.dma_start(en_out[:], en[:])
    nc.compile()
    return nc


def kernel(**inputs):
    from concourse import bass_utils
    in_maps, T_list, G4, e0 = host_prep(inputs)
    key = (tuple(T_list), G4)
    if key not in _CACHE:
        _CACHE[key] = build_kernel(T_list, G4)
    nc = _CACHE[key]
    res = bass_utils.run_bass_kernel_spmd(nc, in_maps, core_ids=list(range(BG)))
    energy = np.zeros(BG, np.float32)
    for b in range(BG):
        energy[b] = res.results[b]['en_out'].reshape(-1)[0] + e0[b]
    return energy
